# revision 4
# baseline (speedup 1.0000x reference)
"""LoRA linear on 8 Trainium2 NeuronCores.

out = x @ (W + A @ B)^T + bias
  x: [4, 4096, 4096] f32, W: [4096, 4096], bias: [4096], A: [4096, 16], B: [16, 4096]

Strategy (column-parallel / d_out-sharded, per the tensor-parallel pattern):
  - Host: Weff = W + A@B (0.1% of total FLOPs), pre-transpose x and Weff so the
    contraction dim lands on SBUF partitions with no on-chip transposes, and
    round both to bf16 (rel err ~3e-3, tolerance is 2e-2).  bf16 runs the PE at
    the same 1 row/cycle as fp32r but halves HBM traffic for the streamed x
    (256 MB -> 128 MB per core), moving the kernel from DMA-bound to
    compute-bound.
  - Each core c: out[:, c*512:(c+1)*512] = x @ WeffT[:, c*512:(c+1)*512] + bias_c.
    WeffT shard (4 MB bf16) stays SBUF-resident; xT streams in m-blocks laid
    out host-side as [mb, p, kt*MB] so each DMA is 128 partitions x 16 KB
    fully contiguous; 32 k-tile matmuls accumulate in PSUM; bias add fused
    into the PSUM->SBUF evacuation.
"""
import numpy as np

import concourse.bacc as bacc
import concourse.mybir as mybir
import concourse.tile as tile
from concourse.bass_utils import run_bass_kernel_spmd

BATCH, SEQ, D = 4, 4096, 4096
M = BATCH * SEQ          # 16384 rows
K = D                    # contraction
N_CORES = 8
OS = D // N_CORES        # 512 output cols per core
KT = K // 128            # 32 k-tiles
MB = 256                 # m-block rows per x stream tile
XBUFS = 3                # x-block double-buffering depth

_f32 = mybir.dt.float32
_bf16 = mybir.dt.bfloat16
_np_bf16 = mybir.dt.np(mybir.dt.bfloat16)

_COMPILED = None


def _build(repeat=1):
    """repeat>1 wraps the compute in a For_i loop that redundantly recomputes
    the same output -- used only for marginal-cost HW timing (the axon
    dispatch floor is ~80ms, far above the ~1ms kernel)."""
    import contextlib
    nc = bacc.Bacc("TRN2", target_bir_lowering=False, debug=False,
                   num_devices=N_CORES)
    # x pre-blocked on host: [mb, p, kt*MB] so each block DMA is 128
    # partitions x (KT*MB) contiguous elements
    xT = nc.dram_tensor("xT", [M // MB, 128, KT * MB], _bf16,
                        kind="ExternalInput").ap()
    wT = nc.dram_tensor("wT", [K, OS], _bf16, kind="ExternalInput").ap()
    bias = nc.dram_tensor("bias", [128, OS], _f32, kind="ExternalInput").ap()
    out = nc.dram_tensor("out", [M, OS], _f32, kind="ExternalOutput").ap()

    with tile.TileContext(nc) as tc:
        with tc.tile_pool(name="w", bufs=1) as wp, \
             tc.tile_pool(name="xb", bufs=XBUFS) as xp, \
             tc.tile_pool(name="ob", bufs=4) as op_, \
             tc.tile_pool(name="ps", bufs=4, space="PSUM") as pp:
            w_sb = []
            for kt in range(KT):
                t = wp.tile([128, OS], _bf16, tag=f"w{kt}")
                nc.sync.dma_start(out=t[:], in_=wT[kt * 128:(kt + 1) * 128, :])
                w_sb.append(t)
            b_sb = wp.tile([128, OS], _f32, tag="bias")
            nc.sync.dma_start(out=b_sb[:], in_=bias)

            loop_cm = (tc.For_i(0, repeat, 1) if repeat > 1
                       else contextlib.nullcontext())
            with loop_cm:
                _emit_body(nc, tc, xp, op_, pp, xT, out, w_sb, b_sb)

    nc.compile()
    return nc


def _emit_body(nc, tc, xp, op_, pp, xT, out, w_sb, b_sb):
    # Two accumulation chains interleaved across two PSUM banks: consecutive
    # matmul instructions target different banks, so each matmul's fill
    # overlaps the previous one's 128-cycle drain (same-bank back-to-back
    # accumulation serializes fill-after-drain, costing +53ns/matmul).
    for mb in range(M // MB):
        xt = xp.tile([128, KT * MB], _bf16, tag="x")
        nc.sync.dma_start(out=xt[:], in_=xT[mb])
        ps = [pp.tile([128, OS], _f32, tag=f"acc{ms}", name=f"ps{ms}")
              for ms in range(MB // 128)]
        for kt in range(KT):
            for ms in range(MB // 128):
                nc.tensor.matmul(
                    ps[ms][:],
                    xt[:, kt * MB + ms * 128:kt * MB + ms * 128 + 128],
                    w_sb[kt][:],
                    start=(kt == 0), stop=(kt == KT - 1))
        for ms in range(MB // 128):
            o_sb = op_.tile([128, OS], _f32, tag="o")
            nc.vector.tensor_add(o_sb[:], ps[ms][:], b_sb[:])
            row = mb * MB + ms * 128
            nc.sync.dma_start(out=out[row:row + 128, :], in_=o_sb[:])


def _compiled():
    global _COMPILED
    if _COMPILED is None:
        _COMPILED = _build()
    return _COMPILED


def _prep_in_maps(x, W, bias, A, B):
    x = np.asarray(x, dtype=np.float32).reshape(M, K)
    W = np.asarray(W, dtype=np.float32)
    bias = np.asarray(bias, dtype=np.float32)
    A = np.asarray(A, dtype=np.float32)
    B = np.asarray(B, dtype=np.float32)

    weff_t = (W + A @ B).T.astype(_np_bf16)  # [K, D] k-major
    x_t = x.T.astype(_np_bf16)               # [K, M] k-major
    # block layout: [mb, p, kt*MB] so each (mb) block is one DMA with
    # 16 KB contiguous per partition
    x_t = np.ascontiguousarray(
        x_t.reshape(KT, 128, M // MB, MB).transpose(2, 1, 0, 3)
    ).reshape(M // MB, 128, KT * MB)

    in_maps = []
    for c in range(N_CORES):
        sl = slice(c * OS, (c + 1) * OS)
        in_maps.append({
            "xT": x_t,
            "wT": np.ascontiguousarray(weff_t[:, sl]),
            "bias": np.tile(bias[sl], (128, 1)),
        })
    return in_maps


def kernel(x, W, bias, A, B):
    nc = _compiled()
    in_maps = _prep_in_maps(x, W, bias, A, B)
    res = run_bass_kernel_spmd(nc, in_maps, core_ids=list(range(N_CORES)),
                               trace=False)
    out = np.concatenate([res.results[c]["out"] for c in range(N_CORES)],
                         axis=1)
    return out.reshape(BATCH, SEQ, D)
